# revision 59
# baseline (speedup 1.0000x reference)
"""BEiT window attention (B=8, N=1024, C=768, 12 heads) on 8 TRN2 NeuronCores.

Sharding: pure data-parallel over batch — one batch element per core, no
collectives. Per-core dataflow (bf16 matmuls, f32 PSUM accumulation):

  qT/kT computed feature-major (transposed) so the attention scores are
  produced directly as S^T (keys on partitions, queries free) and softmax
  needs no on-device transposes.  P = exp(S^T) * E^T with E = exp(rel-pos
  bias) precomputed host-side.  Softmax denominators come from an all-ones
  block appended to the stationary V operand.

  Schedule: one fused software pipeline.  The scalar engine runs ONLY the
  96 exp activations (its hard floor ~1.28us each); PSUM->SBUF copies and
  bias adds run on the vector engine; qkv/proj weight-chunk emissions are
  interleaved into the attention pair loop so the PE array never idles.
  The normalization chain of iteration i (denominator scatter,
  reciprocal, DRAM-bounce broadcast, final scales) is deferred into
  iteration i+1 at fixed checkpoints, so its multi-hop DMA latency rides
  entirely in the shadow of compute; its DMAs issue from the otherwise
  idle gpsimd queue and the final scales run on the gpsimd Q7 cores.  The
  last pair uses an on-chip fast path (PE-array broadcast of 1/Z) to keep
  the tail short.  Weight loads are split so only q0/k0 columns gate the
  first matmuls; E tiles prefetch one iteration ahead.
"""

import sys
import types

import numpy as np
import ml_dtypes

BF16NP = ml_dtypes.bfloat16

P = 128        # partitions
NTOK = 1024    # tokens per batch element
C = 768        # embed dim
NH = 12        # heads
HD = 64        # head dim
NPAIR = 6      # head pairs
NQT = 2        # query tiles of 512
QW = 512       # query tile width
KC = 8         # key chunks of 128
NCORES = 8


def _install_axon_hooks():
    """Register the NTFF profile hook module missing from this image's antenv."""
    if "antenv.axon_hooks" in sys.modules:
        return
    try:
        import antenv  # noqa: F401
        from trn_agent_boot.trn_boot import _ntff_profile_via_ctypes

        mod = types.ModuleType("antenv.axon_hooks")
        mod._hook = _ntff_profile_via_ctypes("/opt/axon/libaxon_pjrt.so")
        mod.get_axon_ntff_profile_hook = lambda: mod._hook
        mod.set_axon_ntff_profile_hook = lambda h: setattr(mod, "_hook", h)
        sys.modules["antenv.axon_hooks"] = mod
    except Exception:
        pass


_BUILD_CACHE = {}


def _build():
    if "nc" in _BUILD_CACHE:
        return _BUILD_CACHE["nc"]

    from contextlib import ExitStack

    import concourse.bass as bass
    import concourse.bacc as bacc
    import concourse.mybir as mybir
    import concourse.tile as tile

    BF = mybir.dt.bfloat16
    F32 = mybir.dt.float32
    AF = mybir.ActivationFunctionType

    nc = bacc.Bacc("TRN2", target_bir_lowering=False, debug=False)

    xT_d = nc.dram_tensor("xT", [C, NTOK], BF, kind="ExternalInput").ap()
    wqkvT_d = nc.dram_tensor("wqkvT", [C, 3 * C], BF, kind="ExternalInput").ap()
    # contiguous copy of just the q0|k0 weight columns, per c-chunk, so the
    # first q/k emissions aren't gated on the full 3.5MB weight load
    wqk0_d = nc.dram_tensor("wqk0", [6, P, 2 * P], BF, kind="ExternalInput").ap()
    qkb_d = nc.dram_tensor("qkb", [P, 12], F32, kind="ExternalInput").ap()
    vb_d = nc.dram_tensor("vb", [1, C], BF, kind="ExternalInput").ap()
    # (pair, qtile, kchunk-pair, key-part, kchunk-in-pair, head-in-pair, q)
    ET_d = nc.dram_tensor(
        "ET", [NPAIR, NQT, KC // 2, P, 2, 2, QW], BF, kind="ExternalInput"
    ).ap()
    pwT_d = nc.dram_tensor("pwT", [C, C], BF, kind="ExternalInput").ap()
    pbT_d = nc.dram_tensor("pbT", [P, 6], F32, kind="ExternalInput").ap()
    out_d = nc.dram_tensor("out", [C, NTOK], BF, kind="ExternalOutput").ap()

    with ExitStack() as ctx:
        tc = ctx.enter_context(tile.TileContext(nc))
        const = ctx.enter_context(tc.tile_pool(name="const", bufs=1))
        spool = ctx.enter_context(tc.tile_pool(name="spool", bufs=2, space="PSUM"))
        opool = ctx.enter_context(tc.tile_pool(name="opool", bufs=2, space="PSUM"))
        epool = ctx.enter_context(tc.tile_pool(name="epool", bufs=8))
        prawp = ctx.enter_context(tc.tile_pool(name="praw", bufs=3))
        pfinp = ctx.enter_context(tc.tile_pool(name="pfin", bufs=16))
        ocpp = ctx.enter_context(tc.tile_pool(name="ocp", bufs=4))
        shp = ctx.enter_context(tc.tile_pool(name="shift", bufs=2))
        sinvp = ctx.enter_context(tc.tile_pool(name="sinv", bufs=2))
        zp = ctx.enter_context(tc.tile_pool(name="z", bufs=2))
        dramp = ctx.enter_context(tc.tile_pool(name="dram", bufs=4, space="DRAM"))
        youtp = ctx.enter_context(tc.tile_pool(name="yout", bufs=2))

        # ---- persistent SBUF tensors ----
        xT_sb = const.tile([P, 6, NTOK], BF)          # x^T, feature-major
        w_sb = const.tile([P, 6, 3 * C], BF)          # qkv_w^T (q cols pre-scaled)
        qk_sb = const.tile([P, 12, NTOK], BF)         # q^T (chunks 0-5), k^T (6-11)
        v_sb = const.tile([P, KC, NH * (HD + 1)], BF)  # 12x[v_h|1] blocks per kchunk
        op_sb = const.tile([P, NPAIR, NTOK], BF)      # normalized O^T, pair-stacked
        pw_sb = const.tile([P, 6, C], BF)             # proj_w^T
        qkb_sb = const.tile([P, 12], F32)
        pb_sb = const.tile([P, 6], F32)
        vb_sb = const.tile([P, C], BF)
        wqk0_sb = const.tile([P, 6, 2 * P], BF)       # q0|k0 columns, early

        # critical input prefix on sync: x fully plus the small q0/k0 column
        # tensor; the full weight stream follows behind
        for c in range(6):
            nc.sync.dma_start(out=xT_sb[:, c, :], in_=xT_d[c * P:(c + 1) * P, :])
            nc.sync.dma_start(
                out=wqk0_sb[:, c, :], in_=wqk0_d[c:c + 1, :, :].rearrange(
                    "c p q -> (c p) q"
                )
            )
        for c in range(6):
            nc.sync.dma_start(out=w_sb[:, c, :], in_=wqkvT_d[c * P:(c + 1) * P, :])
        nc.gpsimd.dma_start(out=qkb_sb[:], in_=qkb_d[:])
        nc.gpsimd.dma_start(out=vb_sb[:], in_=vb_d.broadcast_to((P, C)))
        # pb/pw (projection-only) load later so they don't race the
        # critical x/w prefix for DMA bandwidth
        # ones columns of the [v_h | 1] blocks only
        nc.vector.memset(
            v_sb[:].rearrange("p t (h w) -> p t h w", w=HD + 1)[:, :, :, HD:HD + 1],
            1.0,
        )
        ones_sb = const.tile([P, HD], BF)
        nc.vector.memset(ones_sb[:], 1.0)

        # ---- qkv emission helpers ----
        def emit_qk_half(j, half):
            ps = spool.tile([P, QW], F32, tag="qs")
            sl = slice(half * 512, (half + 1) * 512)
            for c in range(6):
                if j in (0, 6):  # early path: small prefix tensor
                    lhsT = wqk0_sb[:, c, (j // 6) * P:(j // 6 + 1) * P]
                else:
                    lhsT = w_sb[:, c, j * P:(j + 1) * P]
                nc.tensor.matmul(
                    ps[:],
                    lhsT=lhsT,
                    rhs=xT_sb[:, c, sl],
                    start=(c == 0),
                    stop=(c == 5),
                )
            nc.vector.tensor_scalar_add(
                qk_sb[:, j, sl], ps[:], qkb_sb[:, j:j + 1]
            )

        def emit_qk(j):
            emit_qk_half(j, 0)
            emit_qk_half(j, 1)

        def emit_v(t):
            ps = spool.tile([P, NTOK], F32, tag="s")
            for c in range(6):
                for off, width in ((0, 512), (512, 256)):
                    nc.tensor.matmul(
                        ps[:, off:off + width],
                        lhsT=xT_sb[:, c, t * P:(t + 1) * P],
                        rhs=w_sb[:, c, 2 * C + off:2 * C + off + width],
                        start=(c == 0),
                        stop=(c == 5),
                    )
            nc.vector.tensor_add(
                v_sb[:, t, :].rearrange("p (h w) -> p h w", w=HD + 1)[:, :, 0:HD],
                ps[:, 0:C].rearrange("p (h w) -> p h w", w=HD),
                vb_sb[:].rearrange("p (h w) -> p h w", w=HD),
            )

        def issue_e_loads(p, qt):
            tiles = []
            for t in range(4):
                e2 = epool.tile([P, 2 * NTOK], BF, tag="e")
                nc.sync.dma_start(
                    out=e2[:],
                    in_=ET_d[
                        p:p + 1, qt:qt + 1, t:t + 1, :, :, :, :
                    ].rearrange("a b c p d h q -> (a b c p) (d h q)"),
                )
                tiles.append(e2)
            return tiles

        # ---- deferred normalization pipeline ----
        # iteration i's chain runs at fixed checkpoints inside iteration
        # i+1, staggered so every piece's input has landed (DMA payload +
        # semaphore) before its queue reaches it — a premature wait at a
        # queue head stalls every op behind it
        def make_norm(p, qt, ocpA, ocpB):
            state = {}

            def zt():
                t = zp.tile([P, 8], BF, tag="zt")
                nc.gpsimd.dma_start(out=t[:, 0:4], in_=ocpA[HD:HD + 1, :])
                nc.gpsimd.dma_start(out=t[:, 4:8], in_=ocpB[HD:HD + 1, :])
                state["zt"] = t

            def recip():
                zi = zp.tile([P, 8], BF, tag="zi")
                with nc.allow_low_precision(reason="softmax denom fits bf16"):
                    nc.vector.reciprocal(zi[:], state["zt"][:])
                state["zi"] = zi

            def zd():
                zdA = dramp.tile([1, QW], BF, tag="zd")
                zdB = dramp.tile([1, QW], BF, tag="zd")
                nc.sync.dma_start(out=zdA[:], in_=state["zi"][:, 0:4])
                nc.sync.dma_start(out=zdB[:], in_=state["zi"][:, 4:8])
                state["zd"] = (zdA, zdB)

            def sinv():
                zdA, zdB = state["zd"]
                sv = sinvp.tile([P, QW], BF, tag="sv")
                nc.sync.dma_start(
                    out=sv[0:HD, :], in_=zdA[:].broadcast_to((HD, QW))
                )
                nc.sync.dma_start(
                    out=sv[HD:P, :], in_=zdB[:].broadcast_to((HD, QW))
                )
                ocpB2 = shp.tile([P, QW], BF, tag="sh")
                nc.gpsimd.dma_start(out=ocpB2[HD:P, :], in_=ocpB[0:HD, :])
                state["sinv"] = sv
                state["ocpB2"] = ocpB2

            def muls():
                sl = slice(qt * QW, (qt + 1) * QW)
                nc.gpsimd.tensor_mul(
                    op_sb[0:HD, p, sl], ocpA[0:HD, :], state["sinv"][0:HD, :]
                )
                nc.gpsimd.tensor_mul(
                    op_sb[HD:P, p, sl], state["ocpB2"][HD:P, :],
                    state["sinv"][HD:P, :]
                )

            return [zt, recip, zd, sinv, muls]

        # prologue: just enough q/k for the first scores
        emit_qk_half(0, 0)
        emit_qk_half(6, 0)

        iters = [(p, qt) for p in range(NPAIR) for qt in range(NQT)]
        e_tiles = issue_e_loads(0, 0)
        next_e = None
        pending = None
        pend2 = None

        def attnv(pp, ptiles, kc, poA, poB):
            for h in range(2):
                head = 2 * pp + h
                o_ps = poA if h == 0 else poB
                nc.tensor.matmul(
                    o_ps[0:HD + 1, :],
                    lhsT=v_sb[:, kc, (HD + 1) * head:(HD + 1) * (head + 1)],
                    rhs=ptiles[kc][:, h * QW:(h + 1) * QW],
                    start=(kc == 0),
                    stop=(kc == KC - 1),
                )

        def casts(poA, poB):
            ocpA = ocpp.tile([P, QW], BF, tag="ocp")
            ocpB = ocpp.tile([P, QW], BF, tag="ocp")
            nc.vector.tensor_copy(ocpA[0:HD + 1, :], poA[0:HD + 1, :])
            nc.vector.tensor_copy(ocpB[0:HD + 1, :], poB[0:HD + 1, :])
            return ocpA, ocpB

        # CASCADE: iteration i computes scores/exp/multiply for (p, qt)
        # while the tensor engine retires iteration i-1's attn@V from its
        # saved ptiles — attn@V operands are always a full iteration old, so
        # the PE never chases the exp chain and stays in its fast p-state
        prev = None  # (pair, ptiles) of the not-yet-retired iteration
        for it, (p, qt) in enumerate(iters):
            first = it == 0
            if next_e is not None:
                e_tiles = next_e
            # pb/pw arrive before pair 2 starts; plenty ahead of projection
            if it == 2:
                nc.gpsimd.dma_start(out=pb_sb[:], in_=pbT_d[:])
                for c in range(6):
                    nc.gpsimd.dma_start(
                        out=pw_sb[:, c, :], in_=pwT_d[c * P:(c + 1) * P, :]
                    )
            if prev is not None:
                poA = opool.tile([P, QW], F32, tag="o")
                poB = opool.tile([P, QW], F32, tag="o")
            ptiles = []
            for kc in range(KC):
                if first and kc == 4:
                    emit_qk_half(6, 1)  # k chunks 4-7
                s_ps = spool.tile([P, NTOK], F32, tag="s")
                for h in range(2):
                    hh = HD * h
                    nc.tensor.matmul(
                        s_ps[:, h * QW:(h + 1) * QW],
                        lhsT=qk_sb[hh:hh + HD, 6 + p, kc * P:(kc + 1) * P],
                        rhs=qk_sb[hh:hh + HD, p, qt * QW:(qt + 1) * QW],
                        start=True,
                        stop=True,
                    )
                praw = prawp.tile([P, NTOK], BF, tag="pr")
                nc.scalar.activation(out=praw[:], in_=s_ps[:], func=AF.Exp)
                ptile = pfinp.tile([P, NTOK], BF, tag="pf")
                nc.vector.tensor_mul(
                    ptile[:], praw[:],
                    e_tiles[kc // 2][:, (kc % 2) * NTOK:(kc % 2 + 1) * NTOK],
                )
                ptiles.append(ptile)
                if it == 1:
                    emit_v(kc)  # just-in-time V, right before attn@V uses it
                # deferred norm checkpoints, spread across TWO iterations so
                # every piece's input has landed with margin even under DMA
                # timing jitter (a premature queue-head wait stalls the queue)
                if kc == 0 and pend2 is not None:
                    pend2[4]()  # final scales of the norm from 2 back
                if pending is not None:
                    if kc == 1:
                        pending[0]()
                    elif kc == 4:
                        pending[1]()
                    elif kc == 5:
                        pending[2]()
                    elif kc == 7:
                        pending[3]()
                if kc == 6 and it + 1 < len(iters):
                    next_e = issue_e_loads(*iters[it + 1])
                if prev is not None:
                    attnv(prev[0], prev[1], kc, poA, poB)

            if prev is not None:
                ocpA, ocpB = casts(poA, poB)
                pend2 = pending
                pending = make_norm(prev[0], prev[2], ocpA, ocpB)

            if first:
                emit_qk_half(0, 1)  # q of pair 0, queries 512-1023
            elif p == 0:
                emit_qk(1)
                emit_qk(7)
            elif qt == 0 and p < NPAIR - 1:
                emit_qk(p + 1)
            elif qt == 1 and p < NPAIR - 1:
                emit_qk(7 + p)

            prev = (p, ptiles, qt)

        # ---- output projection, split so the last pair's chunk can be
        # grafted on after its normalization lands ----
        proj_ps = {}

        def proj_partial(ec):
            ps = spool.tile([P, NTOK], F32, tag="s")
            proj_ps[ec] = ps
            for pp in range(NPAIR - 1):
                for nt in range(2):
                    sl = slice(nt * 512, (nt + 1) * 512)
                    nc.tensor.matmul(
                        ps[:, sl],
                        lhsT=pw_sb[:, pp, ec * P:(ec + 1) * P],
                        rhs=op_sb[:, pp, sl],
                        start=(pp == 0),
                        stop=False,
                    )

        def proj_finish(ec):
            ps = proj_ps.pop(ec)
            for nt in range(2):
                sl = slice(nt * 512, (nt + 1) * 512)
                nc.tensor.matmul(
                    ps[:, sl],
                    lhsT=pw_sb[:, NPAIR - 1, ec * P:(ec + 1) * P],
                    rhs=op_sb[:, NPAIR - 1, sl],
                    start=False,
                    stop=True,
                )
            y_sb = youtp.tile([P, NTOK], BF)
            nc.vector.tensor_scalar_add(y_sb[:], ps[:], pb_sb[:, ec:ec + 1])
            nc.gpsimd.dma_start(out=out_d[ec * P:(ec + 1) * P, :], in_=y_sb[:])

        # ---- epilogue: retire the last iteration's attn@V ----
        poA = opool.tile([P, QW], F32, tag="o")
        poB = opool.tile([P, QW], F32, tag="o")
        for kc in range(KC):
            if kc == 0 and pend2 is not None:
                pend2[4]()
            if pending is not None:
                if kc == 1:
                    pending[0]()
                elif kc == 2:
                    pending[1]()
                elif kc == 3:
                    pending[2]()
                elif kc == 5:
                    pending[3]()
                elif kc == 7:
                    pending[4]()
            attnv(prev[0], prev[1], kc, poA, poB)
        ocpA, ocpB = casts(poA, poB)
        # last pair: on-chip fast path — broadcast the RAW Z rows across
        # partitions via rank-1 PE matmuls into the now-free attn@V PSUM
        # banks, then full-width reciprocal.  No DRAM bounce, so the tail
        # never queues behind bulk DMA traffic.
        sA = opool.tile([P, QW], F32, tag="o")
        sB = opool.tile([P, QW], F32, tag="o")
        nc.tensor.matmul(
            sA[0:HD, :], lhsT=ones_sb[HD:HD + 1, :],
            rhs=ocpA[HD:HD + 1, :], start=True, stop=True,
        )
        nc.tensor.matmul(
            sB[0:HD, :], lhsT=ones_sb[HD:HD + 1, :],
            rhs=ocpB[HD:HD + 1, :], start=True, stop=True,
        )
        svA = sinvp.tile([P, QW], BF, tag="sv")
        svB = sinvp.tile([P, QW], BF, tag="sv")
        with nc.allow_low_precision(reason="softmax denom fits bf16"):
            nc.vector.reciprocal(svA[0:HD, :], sA[0:HD, :])
            nc.vector.reciprocal(svB[0:HD, :], sB[0:HD, :])
        sl = slice(QW, 2 * QW)
        nc.vector.tensor_mul(
            op_sb[0:HD, NPAIR - 1, sl], ocpA[0:HD, :], svA[0:HD, :]
        )
        stageB = shp.tile([P, QW], BF, tag="sh")
        nc.vector.tensor_mul(stageB[0:HD, :], ocpB[0:HD, :], svB[0:HD, :])
        nc.gpsimd.dma_start(out=op_sb[HD:P, NPAIR - 1, sl], in_=stageB[0:HD, :])

        # ---- output projection ----
        for ec in range(6):
            ps = spool.tile([P, NTOK], F32, tag="s")
            for pp in range(NPAIR):
                for nt in range(2):
                    sl = slice(nt * 512, (nt + 1) * 512)
                    nc.tensor.matmul(
                        ps[:, sl],
                        lhsT=pw_sb[:, pp, ec * P:(ec + 1) * P],
                        rhs=op_sb[:, pp, sl],
                        start=(pp == 0),
                        stop=(pp == NPAIR - 1),
                    )
            y_sb = youtp.tile([P, NTOK], BF)
            nc.vector.tensor_scalar_add(y_sb[:], ps[:], pb_sb[:, ec:ec + 1])
            nc.gpsimd.dma_start(out=out_d[ec * P:(ec + 1) * P, :], in_=y_sb[:])

    nc.compile()
    _BUILD_CACHE["nc"] = nc
    return nc


def _prep_inputs(x, qkv_w, q_bias, v_bias, rel_bias_table, proj_w, proj_b,
                 rel_pos_idx):
    x = np.asarray(x, np.float32)
    qkv_w = np.asarray(qkv_w, np.float32)
    q_bias = np.asarray(q_bias, np.float32)
    v_bias = np.asarray(v_bias, np.float32)
    rel_bias_table = np.asarray(rel_bias_table, np.float32)
    proj_w = np.asarray(proj_w, np.float32)
    proj_b = np.asarray(proj_b, np.float32)
    rel_pos_idx = np.asarray(rel_pos_idx, np.int64)

    scale = HD ** -0.5
    wq = qkv_w[:C] * scale
    wqkvT = np.ascontiguousarray(
        np.concatenate([wq, qkv_w[C:]], axis=0).T
    ).astype(BF16NP)

    # q0|k0 weight columns per c-chunk, contiguous, for the fast prologue
    wqk0 = np.ascontiguousarray(
        np.concatenate(
            [wqkvT.reshape(6, P, 3 * C)[:, :, 0:P],
             wqkvT.reshape(6, P, 3 * C)[:, :, C:C + P]], axis=2
        )
    )

    qk_bias = np.concatenate([q_bias * scale, np.zeros(C, np.float32)])
    qkb = np.ascontiguousarray(qk_bias.reshape(12, P).T)

    vb = v_bias.astype(BF16NP).reshape(1, C)

    # E^T[h, m, n] = exp(bias[h, n, m]); bias[h, n, m] = table[idx[n, m], h]
    A = np.exp(rel_bias_table)[rel_pos_idx]            # (n, m, h)
    ETpre = A.transpose(2, 1, 0)                       # (h, m, n)
    # (pair, hin, kc2, kcin, keypart, qt, q) -> (pair, qt, kc2, keypart,
    # kcin, hin, q): two key-chunks land contiguously per partition per load
    ET = np.ascontiguousarray(
        ETpre.reshape(NPAIR, 2, KC // 2, 2, P, NQT, QW)
        .transpose(0, 5, 2, 4, 3, 1, 6)
    ).astype(BF16NP)

    pwT = np.ascontiguousarray(proj_w.T).astype(BF16NP)
    pbT = np.ascontiguousarray(proj_b.reshape(6, P).T)

    shared = {
        "wqkvT": wqkvT, "wqk0": wqk0, "qkb": qkb, "vb": vb, "ET": ET,
        "pwT": pwT, "pbT": pbT,
    }
    in_maps = []
    xb16 = x.reshape(NCORES, NTOK, C).astype(BF16NP)
    for b in range(NCORES):
        m = dict(shared)
        m["xT"] = np.ascontiguousarray(xb16[b].T)
        in_maps.append(m)
    return in_maps


def _run(inputs, trace=False):
    import time as _time

    _install_axon_hooks()
    from concourse.bass_utils import run_bass_kernel_spmd

    t0 = _time.time()
    nc = _build()
    print(f"[kernel] build+compile: {_time.time() - t0:.1f}s", flush=True)
    t0 = _time.time()
    in_maps = _prep_inputs(**inputs)
    print(f"[kernel] host prep: {_time.time() - t0:.1f}s", flush=True)
    t0 = _time.time()
    res = run_bass_kernel_spmd(
        nc, in_maps, core_ids=list(range(NCORES)), trace=trace
    )
    print(f"[kernel] hw run: {_time.time() - t0:.1f}s", flush=True)
    outs = [np.asarray(res.results[b]["out"]) for b in range(NCORES)]
    y = np.stack([o.astype(np.float32).T.reshape(32, 32, C) for o in outs])
    return y, res


def kernel(**inputs) -> np.ndarray:
    y, _ = _run(inputs, trace=False)
    return y
